# revision 33
# baseline (speedup 1.0000x reference)
"""CRF token-mean NLL on 8 Trainium2 NeuronCores — meet-in-the-middle,
block-diagonal packed forward algorithm.

Math
----
out = sum_b(llh_b / labels_b) / count_nonempty,  llh_b = den_b - num_b.
num (gold path score): cheap host gather (numpy f64).

den_b = log partition via the forward algorithm in probability space:
with E = exp(trans), x_j = softmax(em_j) (host precomputed; shifts a_j
tracked on host), v_j = x_j . (E^T v_{j-1}),  Z_b = w . v_{tail_b},
w = exp(end).

Device structure (per core, 128 seqs, uniform SPMD program):
- State packed [128 partitions, 32 cols]: partition 32a+t = state t of
  chain a; column c = sequence 32a+c.  Weights are 128x128 BLOCK-DIAGONAL
  (4 copies of E) so ONE matmul + ONE DVE multiply advances all 128
  sequences one step.
- Meet in the middle: fwd stream computes v_1..v_511 (steps 1..511); bwd
  stream computes the suffix vector y_m = x_m . (E(y_{m+1}) + w 1[tail=m])
  from m=1022 down to 512 (511 rounds).  The tail "injection" enters as a
  SECOND matmul accumulated into the same PSUM (start/stop flags), keeping
  the DVE chain at one op per round.  Both streams ping-pong PE<->DVE
  concurrently: ~512 sequential rounds instead of 1024.
- Z for tails >= 512: one dot at the meeting point:
  Z_b = (y1_512 + inj_512) . (E^T v_511)  (elementwise mul + ones-matmul).
- Z for tails in [255,511]: w.v over the last 4 rounds of the 8-deep state
  ring captured by a [128,4] w-block matmul every 4 rounds (one round
  lagged, so it runs in PE idle gaps), staged and DMA'd out at the end.
- Renorm every 64 rounds per stream: per-column power-of-two rescale from
  a lagged w.state capture via integer bit ops (clamped to 2^+-30),
  applied as one extra bf16 multiply; host replays bit-exactly.  The
  pipeline is staggered over rounds m-8..m-4 to stay in engine idle gaps.
"""

import numpy as np

B, S, T = 1024, 1024, 32
NCORES = 8
SEQ_PER_CORE = 128
CHAINS = 4
W = 32                      # columns (seqs per chain)
KF = S // 2 - 1             # 511 fwd multiply rounds (steps 1..KF)
NB = S // 2 - 1             # 511 bwd multiply rounds (steps S-2 .. S/2)
MEET = S // 2               # 512: dot uses E^T v_KF and y_{MEET}
RENORM_EVERY = 64
RENORM_LAG = 16             # renorm source precedes its event by this many rounds
OCT = 16                    # state ring-buffer depth
CH = 64                     # rounds per streamed x chunk
FCLAMP = 30                 # renorm factor clamped to 2^+-FCLAMP
CLAMP_LO = np.uint32((127 - FCLAMP) << 23)
CLAMP_HI = np.uint32((127 + FCLAMP) << 23)

# capture rounds: at r = 8k+1 capture w.vf for rounds r-9..r-2 (the final
# capture runs after the loop, covering 504..511)
CAP_ROUNDS = [r for r in range(257, MEET + 2, 8)]
CAP_BASE = CAP_ROUNDS[0] - 9          # first captured round = 248
REN_EVENTS = [m for m in range(RENORM_EVERY, KF + 1 - RENORM_LAG,
                               RENORM_EVERY)]

_PROG_CACHE = {}
TRACE = False
LAST_RESULTS = None


def _build_program():
    import concourse.bacc as bacc
    import concourse.mybir as mybir
    from concourse import tile

    f32 = mybir.dt.float32
    bf16 = mybir.dt.bfloat16
    u32 = mybir.dt.uint32

    nc = bacc.Bacc("TRN2", target_bir_lowering=False, debug=False,
                   enable_asserts=False, num_devices=NCORES)

    P = CHAINS * T  # 128
    # xf holds steps 0..KF (step 0 = initial state); xb/inj as before
    xf_dram = nc.dram_tensor("xf", [P, (KF + 1) * W], bf16,
                             kind="ExternalInput")
    xb_dram = nc.dram_tensor("xb", [P, NB * W], bf16, kind="ExternalInput")
    inj_dram = nc.dram_tensor("inj", [P, (NB + 1) * W], bf16,
                              kind="ExternalInput")
    # combined stationary weights: [wE | wET | ww | wsum]
    wmain_dram = nc.dram_tensor("wmain", [P, 2 * P + 2 * CHAINS], bf16,
                                kind="ExternalInput")
    wbc_dram = nc.dram_tensor("wbc", [CHAINS, P], bf16, kind="ExternalInput")

    ncap = len(CAP_ROUNDS)
    nev = len(REN_EVENTS)
    caps_dram = nc.dram_tensor("caps", [CHAINS, ncap * 8 * W], f32,
                               kind="ExternalOutput")
    rsf_dram = nc.dram_tensor("rsf", [CHAINS, nev * W], f32,
                              kind="ExternalOutput")
    rsb_dram = nc.dram_tensor("rsb", [CHAINS, nev * W], f32,
                              kind="ExternalOutput")
    qdot_dram = nc.dram_tensor("qdot", [CHAINS, W], f32,
                               kind="ExternalOutput")

    nchunks = (KF + 1 + CH - 1) // CH     # fwd chunks: steps 0..KF
    assert nchunks * CH == KF + 1

    with tile.TileContext(nc) as tc:
        with (
            tc.tile_pool(name="const", bufs=1) as constp,
            tc.tile_pool(name="state", bufs=1) as statep,
            tc.tile_pool(name="xs", bufs=3) as xp,
            tc.tile_pool(name="stage", bufs=1) as stgp,
            tc.tile_pool(name="small", bufs=2) as smallp,
            tc.tile_pool(name="psf", bufs=2, space="PSUM") as ppf,
            tc.tile_pool(name="psb", bufs=2, space="PSUM") as ppb,
            tc.tile_pool(name="pscap", bufs=2, space="PSUM") as ppc,
            tc.tile_pool(name="psmisc", bufs=2, space="PSUM") as ppm,
        ):
            # ---- constants (one DMA for the 128-partition stationaries) ----
            wmain = constp.tile([P, 2 * P + 2 * CHAINS], bf16)
            nc.sync.dma_start(wmain[:], wmain_dram[:])
            wE = wmain[:, 0:P]
            wET = wmain[:, P:2 * P]
            ww = wmain[:, 2 * P:2 * P + CHAINS]
            wsum = wmain[:, 2 * P + CHAINS:2 * P + 2 * CHAINS]

            # ---- state ring buffers ----
            vf = statep.tile([P, OCT * W], bf16, name="vf")
            y1 = statep.tile([P, OCT * W], bf16, name="y1")
            nc.vector.memset(y1[:, 0:W], 0.0)

            # ---- x chunk streaming (fwd: step r at chunk r//CH; bwd/inj:
            #      round r at chunk (r-1)//CH) ----
            ftiles = {}
            btiles = {}

            def ensure_fchunk(c, parts=1):
                if c in ftiles or c >= nchunks:
                    return
                lo = c * CH * W
                tf = xp.tile([P, CH * W], bf16, tag="xfc", name="xfc")
                step = CH * W // parts
                for p in range(parts):
                    nc.sync.dma_start(
                        tf[:, p * step:(p + 1) * step],
                        xf_dram[:, lo + p * step:lo + (p + 1) * step])
                ftiles[c] = tf

            def ensure_bchunk(c, parts=1):
                if c in btiles or c * CH >= NB + 1:
                    return
                lo = c * CH * W
                nb_ = min(CH * W, NB * W - lo)
                tb = xp.tile([P, CH * W], bf16, tag="xbc", name="xbc")
                ni = min(CH * W, (NB + 1) * W - lo)
                ti = xp.tile([P, CH * W], bf16, tag="injc", name="injc")
                step = CH * W // parts
                for p in range(parts):
                    ilo, ihi = p * step, min((p + 1) * step, ni)
                    if ilo < ihi:
                        eng = nc.scalar if (c == 0 and p == 0) else nc.gpsimd
                        eng.dma_start(ti[:, ilo:ihi],
                                      inj_dram[:, lo + ilo:lo + ihi])
                    blo, bhi = p * step, min((p + 1) * step, nb_)
                    if blo < bhi:
                        nc.scalar.dma_start(tb[:, blo:bhi],
                                            xb_dram[:, lo + blo:lo + bhi])
                btiles[c] = (tb, ti)

            ensure_fchunk(0, parts=4)
            ensure_bchunk(0, parts=4)
            wbc = constp.tile([CHAINS, P], bf16)
            nc.sync.dma_start(wbc[:], wbc_dram[:])
            ensure_fchunk(1)
            ensure_bchunk(1)

            # ---- staging tiles (filled over the run, DMA'd at the end) ----
            cap_stage = stgp.tile([CHAINS, ncap * 8 * W], f32, name="capst")
            rsf_stage = stgp.tile([CHAINS, nev * W], f32, name="rsfst")
            rsb_stage = stgp.tile([CHAINS, nev * W], f32, name="rsbst")

            fac_f = {}
            fac_b = {}
            g5b_f = {}
            g5b_b = {}
            xs_f = {}
            xs_b = {}

            def slot(r):
                return (r % OCT) * W

            def renorm_bitops(src_psum, stage, ev_idx, g5b_map, m):
                """src [4,W] PSUM -> staged copy; bf16 2^-e clamped factor
                computed on the otherwise-idle GPSIMD from the staged copy."""
                ssl = stage[:, ev_idx * W:(ev_idx + 1) * W]
                nc.scalar.copy(ssl, src_psum[0:CHAINS, :])
                g = smallp.tile([CHAINS, W], f32, tag="g1", name="g1")
                nc.vector.tensor_scalar(
                    g[:].bitcast(u32), ssl.bitcast(u32),
                    int(CLAMP_LO), int(CLAMP_HI),
                    mybir.AluOpType.max, mybir.AluOpType.min)
                g2 = smallp.tile([CHAINS, W], f32, tag="g2", name="g2")
                nc.vector.tensor_scalar(
                    g2[:].bitcast(u32), g[:].bitcast(u32),
                    0x7F800000, 0x7F800000,
                    mybir.AluOpType.bitwise_and,
                    mybir.AluOpType.bitwise_xor)
                g5b = smallp.tile([CHAINS, W], bf16, tag="g5b", name="g5b")
                nc.vector.tensor_scalar_mul(g5b[:], g2[:], 0.5)
                g5b_map[m] = g5b

            def renorm_bc(g5b, fac_map, m):
                pbc = ppm.tile([P, W], f32, tag="misc", name="pbc")
                nc.tensor.matmul(pbc[:], wbc[:], g5b[:])
                fac = smallp.tile([P, W], bf16, tag=f"fac{m % 2}", name="fac")
                nc.scalar.copy(fac[:], pbc[:])
                fac_map[m] = fac

            ev_srcf = {REN_EVENTS[i] - RENORM_LAG: i for i in range(nev)}
            ev_srcb = {REN_EVENTS[i] - RENORM_LAG + 2: i for i in range(nev)}
            ev_bcf = {REN_EVENTS[i] - RENORM_LAG + 4: i for i in range(nev)}
            ev_bcb = {REN_EVENTS[i] - RENORM_LAG + 6: i for i in range(nev)}
            ev_xsf = {REN_EVENTS[i] - 4: i for i in range(nev)}
            ev_xsb = {REN_EVENTS[i] - 3: i for i in range(nev)}
            cap_set = set(CAP_ROUNDS)

            cap_i = 0
            for r in range(1, MEET + 1):
                cf = r // CH if r <= KF else KF // CH
                cb = (r - 1) // CH
                if r == 16:
                    ensure_fchunk(2)
                    ensure_bchunk(2)
                if r % CH == 0:
                    ensure_fchunk(r // CH + 2)
                if (r - 1) % CH == 0:
                    ensure_bchunk(cb + 2)
                tb_c, ti_c = btiles[cb]
                xboff = ((r - 1) % CH) * W

                is_ev = r in REN_EVENTS
                # ---- backward inj matmul first: no data deps, PE can run
                #      it during idle gaps (start=True clears PSUM).  At r=1
                #      the inj DMA is still in flight, so put MMf first. ----
                psb = psf = None
                if r <= NB and r > 1:
                    psb = ppb.tile([P, W], f32, tag="psb", name="psb")
                    nc.tensor.matmul(psb[:], wET, ti_c[:, xboff:xboff + W],
                                     start=True, stop=False)

                # ---- forward matmul ----
                psf = ppf.tile([P, W], f32, tag="psf", name="psf")
                if r == 1:
                    nc.tensor.matmul(psf[:], wE, ftiles[0][:, 0:W])
                    psb = ppb.tile([P, W], f32, tag="psb", name="psb")
                    nc.tensor.matmul(psb[:], wET, ti_c[:, xboff:xboff + W],
                                     start=True, stop=False)
                else:
                    nc.tensor.matmul(psf[:], wE,
                                     vf[:, slot(r - 1):slot(r - 1) + W])

                if r <= NB:
                    # ---- backward state matmul (accumulates onto inj) ----
                    nc.tensor.matmul(psb[:], wET,
                                     y1[:, slot(r - 1):slot(r - 1) + W],
                                     start=False, stop=True)

                # ---- forward multiply ----
                if r <= KF:
                    if is_ev:
                        xfsl = xs_f[r][:]
                    else:
                        xfsl = ftiles[cf][:, (r % CH) * W:(r % CH) * W + W]
                    nc.vector.tensor_mul(vf[:, slot(r):slot(r) + W],
                                         xfsl, psf[:])
                else:
                    # r == MEET: the dot.  y_512 = y1_512 + inj_512
                    ydot = smallp.tile([P, W], bf16, tag="ydot", name="ydot")
                    nc.vector.tensor_add(
                        ydot[:], y1[:, slot(NB):slot(NB) + W],
                        ti_c[:, xboff:xboff + W])
                    qd = smallp.tile([P, W], bf16, tag="qd", name="qd")
                    nc.vector.tensor_mul(qd[:], ydot[:], psf[:])
                    psq = ppm.tile([P, W], f32, tag="misc", name="psq")
                    nc.tensor.matmul(psq[0:CHAINS, :], wsum, qd[:])
                    qst = smallp.tile([CHAINS, W], f32, tag="qst", name="qst")
                    nc.scalar.copy(qst[:], psq[0:CHAINS, :])
                    nc.sync.dma_start(qdot_dram[:], qst[:])

                # ---- backward multiply ----
                if r <= NB:
                    if is_ev:
                        xbsl = xs_b[r][:]
                    else:
                        xbsl = tb_c[:, xboff:xboff + W]
                    nc.vector.tensor_mul(y1[:, slot(r):slot(r) + W],
                                         xbsl, psb[:])

                # ---- captures: w.vf over rounds r-9..r-2 (2-round lag so
                #      the matmul never parks in the in-order PE queue) ----
                if r in cap_set:
                    o0 = ((r - 9) % OCT) * W
                    assert o0 + 8 * W <= OCT * W, r
                    psc = ppc.tile([CHAINS, 8 * W], f32, tag="psc", name="psc")
                    nc.tensor.matmul(psc[:], ww, vf[:, o0:o0 + 8 * W])
                    nc.scalar.copy(
                        cap_stage[:, cap_i * 8 * W:(cap_i + 1) * 8 * W],
                        psc[:])
                    cap_i += 1
                    if cap_i % 8 == 0 or cap_i == ncap:
                        lo = (cap_i - 1) // 8 * 8 * 8 * W
                        hi = cap_i * 8 * W
                        nc.sync.dma_start(caps_dram[:, lo:hi],
                                          cap_stage[:, lo:hi])

                # ---- renorm pipeline (staggered, all reads lagged) ----
                if r in ev_srcf:
                    ei = ev_srcf[r]
                    m = REN_EVENTS[ei]
                    src = ppm.tile([P, W], f32, tag="misc", name="rsrc")
                    nc.tensor.matmul(src[0:CHAINS, :], ww,
                                     vf[:, slot(r - 2):slot(r - 2) + W])
                    renorm_bitops(src, rsf_stage, ei, g5b_f, m)
                if r in ev_bcf:
                    m = REN_EVENTS[ev_bcf[r]]
                    renorm_bc(g5b_f[m], fac_f, m)
                if r in ev_srcb:
                    ei = ev_srcb[r]
                    m = REN_EVENTS[ei]
                    src = ppm.tile([P, W], f32, tag="misc", name="rsrcb")
                    nc.tensor.matmul(src[0:CHAINS, :], ww,
                                     y1[:, slot(r - 2):slot(r - 2) + W])
                    renorm_bitops(src, rsb_stage, ei, g5b_b, m)
                if r in ev_bcb:
                    m = REN_EVENTS[ev_bcb[r]]
                    renorm_bc(g5b_b[m], fac_b, m)
                if r in ev_xsf:
                    m = REN_EVENTS[ev_xsf[r]]
                    mc = m // CH
                    xt = smallp.tile([P, W], bf16, tag="xsf", name="xsf")
                    nc.vector.tensor_mul(
                        xt[:], ftiles[mc][:, (m % CH) * W:(m % CH) * W + W],
                        fac_f[m][:])
                    xs_f[m] = xt
                if r in ev_xsb:
                    m = REN_EVENTS[ev_xsb[r]]
                    mcb = (m - 1) // CH
                    xob = ((m - 1) % CH) * W
                    xt = smallp.tile([P, W], bf16, tag="xsb", name="xsb")
                    nc.vector.tensor_mul(xt[:], btiles[mcb][0][:, xob:xob + W],
                                         fac_b[m][:])
                    xs_b[m] = xt
                if r == REN_EVENTS[-1] + 8:
                    # all renorm sources staged; ship them overlapped
                    nc.sync.dma_start(rsf_dram[:], rsf_stage[:])
                    nc.sync.dma_start(rsb_dram[:], rsb_stage[:])

            # final capture: rounds 504..511 (octs contiguous at slot 8)
            o0 = ((MEET + 1 - 9) % OCT) * W
            psc = ppc.tile([CHAINS, 8 * W], f32, tag="psc", name="psc")
            nc.tensor.matmul(psc[:], ww, vf[:, o0:o0 + 8 * W])
            cap_fin = smallp.tile([CHAINS, 8 * W], f32, tag="capfin",
                                  name="capfin")
            nc.scalar.copy(cap_fin[:], psc[:])
            nc.sync.dma_start(caps_dram[:, (ncap - 1) * 8 * W:ncap * 8 * W],
                              cap_fin[:])

    nc.compile()
    return nc


def _get_program():
    if "p" not in _PROG_CACHE:
        _PROG_CACHE["p"] = _build_program()
    return _PROG_CACHE["p"]


def _host_prep(em, startt):
    """x = softmax over tags (start folded into step 0); a = log shifts."""
    b, s_len, t = em.shape
    x = em.astype(np.float32, copy=True)
    x[:, 0, :] += startt.astype(np.float32)
    mx = x.max(axis=2)
    x -= mx[:, :, None]
    np.exp(x, out=x)
    ssum = x.sum(axis=2)
    x /= ssum[:, :, None]
    a = mx.astype(np.float64) + np.log(ssum.astype(np.float64))
    return x, a


def _pack_core(xc):
    """[128, S, T] -> [128P, S*W] packed: partition 32a+t, col (r*W + c)."""
    arr = xc.reshape(CHAINS, W, S, T).transpose(0, 3, 2, 1)  # [a, t, r, c]
    return np.ascontiguousarray(arr).reshape(CHAINS * T, S * W)


def _device_inputs(x, trans, endt, tails):
    import ml_dtypes
    bf16 = ml_dtypes.bfloat16
    P = CHAINS * T
    with np.errstate(under="ignore"):
        E = np.exp(trans.astype(np.float64)).astype(np.float32)
        wvec = np.exp(endt.astype(np.float64)).astype(np.float32)
    wmain = np.zeros((P, 2 * P + 2 * CHAINS), np.float32)
    wbc = np.zeros((CHAINS, P), np.float32)
    for a in range(CHAINS):
        sl = slice(a * T, (a + 1) * T)
        wmain[sl, a * T:(a + 1) * T] = E
        wmain[sl, P + a * T:P + (a + 1) * T] = E.T
        wmain[sl, 2 * P + a] = wvec
        wmain[sl, 2 * P + CHAINS + a] = 1.0
        wbc[a, sl] = 1.0
    wmain = wmain.astype(bf16)
    wbc = wbc.astype(bf16)

    in_maps = []
    for core in range(NCORES):
        seqs = slice(core * SEQ_PER_CORE, (core + 1) * SEQ_PER_CORE)
        xc = x[seqs]                       # [128, S, T] f32
        tl = tails[seqs]                   # [128]
        packed = _pack_core(xc)            # [128, S*W] f32, col r*W+c
        p3 = packed.reshape(CHAINS * T, S, W)
        # fwd: steps 0..KF (step 0 = initial state)
        xf = np.ascontiguousarray(
            p3[:, 0:KF + 1]).reshape(CHAINS * T, (KF + 1) * W).astype(bf16)
        # bwd round j -> step S-1-j (j=1..NB: steps S-2 .. MEET)
        steps_b = np.arange(S - 2, MEET - 1, -1)
        xb = np.ascontiguousarray(
            p3[:, steps_b]).reshape(CHAINS * T, NB * W).astype(bf16)
        # inj tiles: round j uses inj_{S-j}; tile NB+1 = inj_{MEET}
        injv = xc * wvec[None, None, :]    # [128, S, T]
        mask_t = np.zeros((SEQ_PER_CORE, S), np.float32)
        mask_t[np.arange(SEQ_PER_CORE), tl] = 1.0
        injv = injv * mask_t[:, :, None]
        pinj = _pack_core(injv).reshape(CHAINS * T, S, W)
        steps_i = np.concatenate([np.arange(S - 1, MEET, -1), [MEET]])
        inj = np.ascontiguousarray(
            pinj[:, steps_i]).reshape(CHAINS * T, (NB + 1) * W).astype(bf16)
        in_maps.append({
            "xf": xf, "xb": xb, "inj": inj, "wmain": wmain, "wbc": wbc,
        })
    return in_maps


def _exp_factor(src):
    """Replay the device's clamped power-of-two renorm factor (f64)."""
    bits = np.ascontiguousarray(src.astype(np.float32)).view(np.uint32)
    bits = np.minimum(np.maximum(bits, CLAMP_LO), CLAMP_HI)
    gbits = (bits & np.uint32(0x7F800000)) ^ np.uint32(0x7F800000)
    return gbits.view(np.float32).astype(np.float64) * 0.5


def _denominators(res, a, tails):
    """Per-seq log partition from device outputs (f64 host replay)."""
    big_a = np.cumsum(a, axis=1)          # [B, S]
    nev = len(REN_EVENTS)
    ncap = len(CAP_ROUNDS)
    mvec = np.array(REN_EVENTS)           # event rounds [nev]
    den = np.zeros(B, np.float64)
    for core in range(NCORES):
        r = res.results[core]
        sl = slice(core * SEQ_PER_CORE, (core + 1) * SEQ_PER_CORE)
        t_b = tails[sl]                                    # [128]
        # [CHAINS, nev, W] -> [nev, 128]
        rsf = r["rsf"].astype(np.float64).reshape(CHAINS, nev, W)
        rsb = r["rsb"].astype(np.float64).reshape(CHAINS, nev, W)
        rsf = np.moveaxis(rsf, 1, 0).reshape(nev, SEQ_PER_CORE)
        rsb = np.moveaxis(rsb, 1, 0).reshape(nev, SEQ_PER_CORE)
        caps = r["caps"].astype(np.float64).reshape(CHAINS, ncap * 8, W)
        caps = caps.transpose(1, 0, 2).reshape(ncap * 8, SEQ_PER_CORE)
        qd = r["qdot"].astype(np.float64).reshape(SEQ_PER_CORE)

        lf = -np.log(_exp_factor(rsf))                     # [nev, 128]
        lb = -np.log(_exp_factor(rsb))
        long = t_b >= MEET
        # fwd offsets: all events for long; m <= tail for short
        use_f = long[None, :] | (mvec[:, None] <= t_b[None, :])
        off = np.sum(np.where(use_f, lf, 0.0), axis=0)
        # bwd offsets (long only): event processes step S-1-m
        use_b = long[None, :] & ((S - 1 - mvec)[:, None] < t_b[None, :])
        off += np.sum(np.where(use_b, lb, 0.0), axis=0)

        z_long = np.log(np.maximum(qd, 1e-300))
        idx = np.clip(t_b - CAP_BASE, 0, ncap * 8 - 1)
        z_short = np.log(np.maximum(caps[idx, np.arange(SEQ_PER_CORE)],
                                    1e-300))
        bidx = np.arange(SEQ_PER_CORE)
        den[sl] = (np.where(long, z_long, z_short)
                   + big_a[sl][bidx, t_b] + off)
    return den


def _numerator(em, tags, mask, startt, trans, endt):
    bsz, s_len, _ = em.shape
    tags = tags.astype(np.int64)
    ar = np.arange(s_len)
    bidx = np.arange(bsz)
    head = np.min(np.where(mask, ar[None, :], s_len - 1), axis=1)
    tail = np.max(ar[None, :] * mask, axis=1)
    nonempty = mask.sum(axis=1) != 0
    cond = mask[:, 1:] & (head[:, None] != ar[None, 1:])
    head_tags = tags[bidx, head]
    tail_tags = tags[bidx, tail]
    em64 = em.astype(np.float64)
    em_tag = np.take_along_axis(em64, tags[:, :, None], axis=2)[:, :, 0]
    trans_step = trans.astype(np.float64)[tags[:, :-1], tags[:, 1:]]
    num = (startt.astype(np.float64)[head_tags]
           + em_tag[bidx, head]
           + np.sum(np.where(cond, trans_step + em_tag[:, 1:], 0.0), axis=1)
           + endt.astype(np.float64)[tail_tags])
    return np.where(nonempty, num, 0.0)


def _finalize(den, num, mask):
    llh = den - num
    labels = mask.sum(axis=1).astype(np.float64)
    eps = 1e-6
    out = np.sum(llh / (labels + eps)) / (np.sum(labels != 0) + eps)
    return np.asarray(out, dtype=np.float32)


def kernel(**inputs):
    from concourse.bass_utils import run_bass_kernel_spmd

    em = np.asarray(inputs["emissions"], dtype=np.float32)
    tags = np.asarray(inputs["tags"])
    mask = np.asarray(inputs["mask"]).astype(bool)
    startt = np.asarray(inputs["start_transitions"], dtype=np.float32)
    trans = np.asarray(inputs["transitions"], dtype=np.float32)
    endt = np.asarray(inputs["end_transitions"], dtype=np.float32)
    bsz, s_len, t = em.shape
    assert (bsz, s_len, t) == (B, S, T), (bsz, s_len, t)

    ar = np.arange(s_len)
    tails = np.max(ar[None, :] * mask, axis=1)  # [B]

    x, a = _host_prep(em, startt)
    nc = _get_program()
    in_maps = _device_inputs(x, trans, endt, tails)
    res = run_bass_kernel_spmd(nc, in_maps, core_ids=list(range(NCORES)),
                               trace=TRACE)
    global LAST_RESULTS
    LAST_RESULTS = res

    den = _denominators(res, a, tails)
    num = _numerator(em, tags, mask, startt, trans, endt)
    return _finalize(den, num, mask)


# revision 38
# speedup vs baseline: 1.0092x; 1.0092x over previous
"""CRF token-mean NLL on 8 Trainium2 NeuronCores — meet-in-the-middle,
block-diagonal packed forward algorithm.

Math
----
out = sum_b(llh_b / labels_b) / count_nonempty,  llh_b = den_b - num_b.
num (gold path score): cheap host gather (numpy f64).

den_b = log partition via the forward algorithm in probability space:
with E = exp(trans), x_j = softmax(em_j) (host precomputed; shifts a_j
tracked on host), v_j = x_j . (E^T v_{j-1}),  Z_b = w . v_{tail_b},
w = exp(end).

Device structure (per core, 128 seqs, uniform SPMD program):
- State packed [128 partitions, 32 cols]: partition 32a+t = state t of
  chain a; column c = sequence 32a+c.  Weights are 128x128 BLOCK-DIAGONAL
  (4 copies of E) so ONE matmul + ONE DVE multiply advances all 128
  sequences one step.
- Meet in the middle: fwd stream computes v_1..v_511 (steps 1..511); bwd
  stream computes the suffix vector y_m = x_m . (E(y_{m+1}) + w 1[tail=m])
  from m=1022 down to 512 (511 rounds).  The tail "injection" enters as a
  SECOND matmul accumulated into the same PSUM (start/stop flags), keeping
  the DVE chain at one op per round.  Both streams ping-pong PE<->DVE
  concurrently: ~512 sequential rounds instead of 1024.
- Z for tails >= 512: one dot at the meeting point:
  Z_b = (y1_512 + inj_512) . (E^T v_511)  (elementwise mul + ones-matmul).
- Z for tails in [255,511]: w.v over the last 4 rounds of the 8-deep state
  ring captured by a [128,4] w-block matmul every 4 rounds (one round
  lagged, so it runs in PE idle gaps), staged and DMA'd out at the end.
- Renorm every 64 rounds per stream: per-column power-of-two rescale from
  a lagged w.state capture via integer bit ops (clamped to 2^+-30),
  applied as one extra bf16 multiply; host replays bit-exactly.  The
  pipeline is staggered over rounds m-8..m-4 to stay in engine idle gaps.
"""

import numpy as np

B, S, T = 1024, 1024, 32
NCORES = 8
SEQ_PER_CORE = 128
CHAINS = 4
W = 32                      # columns (seqs per chain)
KF = S // 2 - 1             # 511 fwd multiply rounds (steps 1..KF)
NB = S // 2 - 1             # 511 bwd multiply rounds (steps S-2 .. S/2)
MEET = S // 2               # 512: dot uses E^T v_KF and y_{MEET}
RENORM_EVERY = 64
RENORM_LAG = 16             # renorm source precedes its event by this many rounds
OCT = 16                    # state ring-buffer depth
CH = 64                     # rounds per streamed x chunk
FCLAMP = 30                 # renorm factor clamped to 2^+-FCLAMP
CLAMP_LO = np.uint32((127 - FCLAMP) << 23)
CLAMP_HI = np.uint32((127 + FCLAMP) << 23)

# capture rounds: at r (mult of 8) capture w.vf for rounds r-8..r-1
CAP_ROUNDS = [r for r in range(256, MEET + 1, 8)]
CAP_BASE = CAP_ROUNDS[0] - 8          # first captured round = 248
REN_EVENTS = [m for m in range(RENORM_EVERY, KF + 1 - RENORM_LAG,
                               RENORM_EVERY)]

_PROG_CACHE = {}
TRACE = False
LAST_RESULTS = None


def _build_program():
    import concourse.bacc as bacc
    import concourse.mybir as mybir
    from concourse import tile

    f32 = mybir.dt.float32
    bf16 = mybir.dt.bfloat16
    u32 = mybir.dt.uint32

    nc = bacc.Bacc("TRN2", target_bir_lowering=False, debug=False,
                   enable_asserts=False, num_devices=NCORES)

    P = CHAINS * T  # 128
    # xf holds steps 0..KF (step 0 = initial state); xb/inj as before
    xf_dram = nc.dram_tensor("xf", [P, (KF + 1) * W], bf16,
                             kind="ExternalInput")
    xb_dram = nc.dram_tensor("xb", [P, NB * W], bf16, kind="ExternalInput")
    inj_dram = nc.dram_tensor("inj", [P, (NB + 1) * W], bf16,
                              kind="ExternalInput")
    # combined stationary weights: [wE | wET | ww | wsum]
    wmain_dram = nc.dram_tensor("wmain", [P, 2 * P + 2 * CHAINS], bf16,
                                kind="ExternalInput")
    wbc_dram = nc.dram_tensor("wbc", [CHAINS, P], bf16, kind="ExternalInput")

    ncap = len(CAP_ROUNDS)
    nev = len(REN_EVENTS)
    caps_dram = nc.dram_tensor("caps", [CHAINS, ncap * 8 * W], f32,
                               kind="ExternalOutput")
    rsf_dram = nc.dram_tensor("rsf", [CHAINS, nev * W], f32,
                              kind="ExternalOutput")
    rsb_dram = nc.dram_tensor("rsb", [CHAINS, nev * W], f32,
                              kind="ExternalOutput")
    qdot_dram = nc.dram_tensor("qdot", [CHAINS, W], f32,
                               kind="ExternalOutput")

    nchunks = (KF + 1 + CH - 1) // CH     # fwd chunks: steps 0..KF
    assert nchunks * CH == KF + 1

    with tile.TileContext(nc) as tc:
        with (
            tc.tile_pool(name="const", bufs=1) as constp,
            tc.tile_pool(name="state", bufs=1) as statep,
            tc.tile_pool(name="xs", bufs=3) as xp,
            tc.tile_pool(name="stage", bufs=1) as stgp,
            tc.tile_pool(name="small", bufs=2) as smallp,
            tc.tile_pool(name="psf", bufs=2, space="PSUM") as ppf,
            tc.tile_pool(name="psb", bufs=2, space="PSUM") as ppb,
            tc.tile_pool(name="pscap", bufs=2, space="PSUM") as ppc,
            tc.tile_pool(name="psmisc", bufs=2, space="PSUM") as ppm,
        ):
            # ---- constants (one DMA for the 128-partition stationaries) ----
            wmain = constp.tile([P, 2 * P + 2 * CHAINS], bf16)
            nc.sync.dma_start(wmain[:], wmain_dram[:])
            wE = wmain[:, 0:P]
            wET = wmain[:, P:2 * P]
            ww = wmain[:, 2 * P:2 * P + CHAINS]
            wsum = wmain[:, 2 * P + CHAINS:2 * P + 2 * CHAINS]

            # ---- state ring buffers ----
            vf = statep.tile([P, OCT * W], bf16, name="vf")
            y1 = statep.tile([P, OCT * W], bf16, name="y1")
            nc.vector.memset(y1[:, 0:W], 0.0)

            # ---- x chunk streaming (fwd: step r at chunk r//CH; bwd/inj:
            #      round r at chunk (r-1)//CH) ----
            ftiles = {}
            btiles = {}

            def ensure_fchunk(c, parts=1):
                if c in ftiles or c >= nchunks:
                    return
                lo = c * CH * W
                tf = xp.tile([P, CH * W], bf16, tag="xfc", name="xfc")
                step = CH * W // parts
                for p in range(parts):
                    nc.sync.dma_start(
                        tf[:, p * step:(p + 1) * step],
                        xf_dram[:, lo + p * step:lo + (p + 1) * step])
                ftiles[c] = tf

            def ensure_bchunk(c, parts=1):
                if c in btiles or c * CH >= NB + 1:
                    return
                lo = c * CH * W
                nb_ = min(CH * W, NB * W - lo)
                tb = xp.tile([P, CH * W], bf16, tag="xbc", name="xbc")
                ni = min(CH * W, (NB + 1) * W - lo)
                ti = xp.tile([P, CH * W], bf16, tag="injc", name="injc")
                step = CH * W // parts
                for p in range(parts):
                    blo, bhi = p * step, min((p + 1) * step, nb_)
                    if blo < bhi:
                        nc.scalar.dma_start(tb[:, blo:bhi],
                                            xb_dram[:, lo + blo:lo + bhi])
                    ilo, ihi = p * step, min((p + 1) * step, ni)
                    if ilo < ihi:
                        nc.gpsimd.dma_start(ti[:, ilo:ihi],
                                            inj_dram[:, lo + ilo:lo + ihi])
                btiles[c] = (tb, ti)

            ensure_fchunk(0, parts=4)
            ensure_bchunk(0, parts=4)
            wbc = constp.tile([CHAINS, P], bf16)
            nc.sync.dma_start(wbc[:], wbc_dram[:])
            ensure_fchunk(1)
            ensure_bchunk(1)

            # ---- staging tiles (filled over the run, DMA'd at the end) ----
            cap_stage = stgp.tile([CHAINS, ncap * 8 * W], f32, name="capst")
            rsf_stage = stgp.tile([CHAINS, nev * W], f32, name="rsfst")
            rsb_stage = stgp.tile([CHAINS, nev * W], f32, name="rsbst")

            fac_f = {}
            fac_b = {}
            g5b_f = {}
            g5b_b = {}
            xs_f = {}
            xs_b = {}

            def slot(r):
                return (r % OCT) * W

            def renorm_bitops(src_psum, stage, ev_idx, g5b_map, m):
                """src [4,W] PSUM -> staged copy; bf16 2^-e clamped factor
                computed on the otherwise-idle GPSIMD from the staged copy."""
                ssl = stage[:, ev_idx * W:(ev_idx + 1) * W]
                nc.scalar.copy(ssl, src_psum[0:CHAINS, :])
                g = smallp.tile([CHAINS, W], f32, tag="g1", name="g1")
                nc.vector.tensor_scalar(
                    g[:].bitcast(u32), ssl.bitcast(u32),
                    int(CLAMP_LO), int(CLAMP_HI),
                    mybir.AluOpType.max, mybir.AluOpType.min)
                g2 = smallp.tile([CHAINS, W], f32, tag="g2", name="g2")
                nc.vector.tensor_scalar(
                    g2[:].bitcast(u32), g[:].bitcast(u32),
                    0x7F800000, 0x7F800000,
                    mybir.AluOpType.bitwise_and,
                    mybir.AluOpType.bitwise_xor)
                g5b = smallp.tile([CHAINS, W], bf16, tag="g5b", name="g5b")
                nc.vector.tensor_scalar_mul(g5b[:], g2[:], 0.5)
                g5b_map[m] = g5b

            def renorm_bc(g5b, fac_map, m):
                pbc = ppm.tile([P, W], f32, tag="misc", name="pbc")
                nc.tensor.matmul(pbc[:], wbc[:], g5b[:])
                fac = smallp.tile([P, W], bf16, tag=f"fac{m % 2}", name="fac")
                nc.scalar.copy(fac[:], pbc[:])
                fac_map[m] = fac

            ev_srcf = {REN_EVENTS[i] - RENORM_LAG: i for i in range(nev)}
            ev_srcb = {REN_EVENTS[i] - RENORM_LAG + 2: i for i in range(nev)}
            ev_bcf = {REN_EVENTS[i] - RENORM_LAG + 4: i for i in range(nev)}
            ev_bcb = {REN_EVENTS[i] - RENORM_LAG + 6: i for i in range(nev)}
            ev_xsf = {REN_EVENTS[i] - 4: i for i in range(nev)}
            ev_xsb = {REN_EVENTS[i] - 3: i for i in range(nev)}
            cap_set = set(CAP_ROUNDS)

            cap_i = 0
            for r in range(1, MEET + 1):
                cf = r // CH if r <= KF else KF // CH
                cb = (r - 1) // CH
                if r == 16:
                    ensure_fchunk(2)
                    ensure_bchunk(2)
                if r % CH == 0:
                    ensure_fchunk(r // CH + 2)
                if (r - 1) % CH == 0:
                    ensure_bchunk(cb + 2)
                tb_c, ti_c = btiles[cb]
                xboff = ((r - 1) % CH) * W

                is_ev = r in REN_EVENTS
                # ---- backward inj matmul first: no data deps, PE can run
                #      it during idle gaps (start=True clears PSUM) ----
                if r <= NB:
                    psb = ppb.tile([P, W], f32, tag="psb", name="psb")
                    nc.tensor.matmul(psb[:], wET, ti_c[:, xboff:xboff + W],
                                     start=True, stop=False)

                # ---- forward matmul ----
                psf = ppf.tile([P, W], f32, tag="psf", name="psf")
                if r == 1:
                    nc.tensor.matmul(psf[:], wE, ftiles[0][:, 0:W])
                else:
                    nc.tensor.matmul(psf[:], wE,
                                     vf[:, slot(r - 1):slot(r - 1) + W])

                if r <= NB:
                    # ---- backward state matmul (accumulates onto inj) ----
                    nc.tensor.matmul(psb[:], wET,
                                     y1[:, slot(r - 1):slot(r - 1) + W],
                                     start=False, stop=True)

                # ---- forward multiply ----
                if r <= KF:
                    if is_ev:
                        xfsl = xs_f[r][:]
                    else:
                        xfsl = ftiles[cf][:, (r % CH) * W:(r % CH) * W + W]
                    nc.vector.tensor_mul(vf[:, slot(r):slot(r) + W],
                                         xfsl, psf[:])
                else:
                    # r == MEET: the dot.  y_512 = y1_512 + inj_512
                    ydot = smallp.tile([P, W], bf16, tag="ydot", name="ydot")
                    nc.vector.tensor_add(
                        ydot[:], y1[:, slot(NB):slot(NB) + W],
                        ti_c[:, xboff:xboff + W])
                    qd = smallp.tile([P, W], bf16, tag="qd", name="qd")
                    nc.vector.tensor_mul(qd[:], ydot[:], psf[:])
                    psq = ppm.tile([P, W], f32, tag="misc", name="psq")
                    nc.tensor.matmul(psq[0:CHAINS, :], wsum, qd[:])
                    qst = smallp.tile([CHAINS, W], f32, tag="qst", name="qst")
                    nc.scalar.copy(qst[:], psq[0:CHAINS, :])
                    nc.sync.dma_start(qdot_dram[:], qst[:])

                # ---- backward multiply ----
                if r <= NB:
                    if is_ev:
                        xbsl = xs_b[r][:]
                    else:
                        xbsl = tb_c[:, xboff:xboff + W]
                    nc.vector.tensor_mul(y1[:, slot(r):slot(r) + W],
                                         xbsl, psb[:])

                # ---- captures: w.vf over rounds r-8..r-1 (lagged) ----
                if r in cap_set:
                    o0 = ((r - 8) % OCT) * W
                    assert o0 + 8 * W <= OCT * W, r
                    psc = ppc.tile([CHAINS, 8 * W], f32, tag="psc", name="psc")
                    nc.tensor.matmul(psc[:], ww, vf[:, o0:o0 + 8 * W])
                    nc.scalar.copy(
                        cap_stage[:, cap_i * 8 * W:(cap_i + 1) * 8 * W],
                        psc[:])
                    cap_i += 1
                    if cap_i % 8 == 0 or cap_i == ncap:
                        lo = (cap_i - 1) // 8 * 8 * 8 * W
                        hi = cap_i * 8 * W
                        nc.sync.dma_start(caps_dram[:, lo:hi],
                                          cap_stage[:, lo:hi])

                # ---- renorm pipeline (staggered, all reads lagged) ----
                if r in ev_srcf:
                    ei = ev_srcf[r]
                    m = REN_EVENTS[ei]
                    src = ppm.tile([P, W], f32, tag="misc", name="rsrc")
                    nc.tensor.matmul(src[0:CHAINS, :], ww,
                                     vf[:, slot(r - 2):slot(r - 2) + W])
                    renorm_bitops(src, rsf_stage, ei, g5b_f, m)
                if r in ev_bcf:
                    m = REN_EVENTS[ev_bcf[r]]
                    renorm_bc(g5b_f[m], fac_f, m)
                if r in ev_srcb:
                    ei = ev_srcb[r]
                    m = REN_EVENTS[ei]
                    src = ppm.tile([P, W], f32, tag="misc", name="rsrcb")
                    nc.tensor.matmul(src[0:CHAINS, :], ww,
                                     y1[:, slot(r - 2):slot(r - 2) + W])
                    renorm_bitops(src, rsb_stage, ei, g5b_b, m)
                if r in ev_bcb:
                    m = REN_EVENTS[ev_bcb[r]]
                    renorm_bc(g5b_b[m], fac_b, m)
                if r in ev_xsf:
                    m = REN_EVENTS[ev_xsf[r]]
                    mc = m // CH
                    xt = smallp.tile([P, W], bf16, tag="xsf", name="xsf")
                    nc.vector.tensor_mul(
                        xt[:], ftiles[mc][:, (m % CH) * W:(m % CH) * W + W],
                        fac_f[m][:])
                    xs_f[m] = xt
                if r in ev_xsb:
                    m = REN_EVENTS[ev_xsb[r]]
                    mcb = (m - 1) // CH
                    xob = ((m - 1) % CH) * W
                    xt = smallp.tile([P, W], bf16, tag="xsb", name="xsb")
                    nc.vector.tensor_mul(xt[:], btiles[mcb][0][:, xob:xob + W],
                                         fac_b[m][:])
                    xs_b[m] = xt
                if r == REN_EVENTS[-1] + 8:
                    # all renorm sources staged; ship them overlapped
                    nc.sync.dma_start(rsf_dram[:], rsf_stage[:])
                    nc.sync.dma_start(rsb_dram[:], rsb_stage[:])

    nc.compile()
    return nc


def _get_program():
    if "p" not in _PROG_CACHE:
        _PROG_CACHE["p"] = _build_program()
    return _PROG_CACHE["p"]


def _host_prep(em, startt):
    """x = softmax over tags (start folded into step 0); a = log shifts."""
    b, s_len, t = em.shape
    x = em.astype(np.float32, copy=True)
    x[:, 0, :] += startt.astype(np.float32)
    mx = x.max(axis=2)
    x -= mx[:, :, None]
    np.exp(x, out=x)
    ssum = x.sum(axis=2)
    x /= ssum[:, :, None]
    a = mx.astype(np.float64) + np.log(ssum.astype(np.float64))
    return x, a


def _pack_core(xc):
    """[128, S, T] -> [128P, S*W] packed: partition 32a+t, col (r*W + c)."""
    arr = xc.reshape(CHAINS, W, S, T).transpose(0, 3, 2, 1)  # [a, t, r, c]
    return np.ascontiguousarray(arr).reshape(CHAINS * T, S * W)


def _device_inputs(x, trans, endt, tails):
    import ml_dtypes
    bf16 = ml_dtypes.bfloat16
    P = CHAINS * T
    with np.errstate(under="ignore"):
        E = np.exp(trans.astype(np.float64)).astype(np.float32)
        wvec = np.exp(endt.astype(np.float64)).astype(np.float32)
    wmain = np.zeros((P, 2 * P + 2 * CHAINS), np.float32)
    wbc = np.zeros((CHAINS, P), np.float32)
    for a in range(CHAINS):
        sl = slice(a * T, (a + 1) * T)
        wmain[sl, a * T:(a + 1) * T] = E
        wmain[sl, P + a * T:P + (a + 1) * T] = E.T
        wmain[sl, 2 * P + a] = wvec
        wmain[sl, 2 * P + CHAINS + a] = 1.0
        wbc[a, sl] = 1.0
    wmain = wmain.astype(bf16)
    wbc = wbc.astype(bf16)

    in_maps = []
    for core in range(NCORES):
        seqs = slice(core * SEQ_PER_CORE, (core + 1) * SEQ_PER_CORE)
        xc = x[seqs]                       # [128, S, T] f32
        tl = tails[seqs]                   # [128]
        packed = _pack_core(xc)            # [128, S*W] f32, col r*W+c
        p3 = packed.reshape(CHAINS * T, S, W)
        # fwd: steps 0..KF (step 0 = initial state)
        xf = np.ascontiguousarray(
            p3[:, 0:KF + 1]).reshape(CHAINS * T, (KF + 1) * W).astype(bf16)
        # bwd round j -> step S-1-j (j=1..NB: steps S-2 .. MEET)
        steps_b = np.arange(S - 2, MEET - 1, -1)
        xb = np.ascontiguousarray(
            p3[:, steps_b]).reshape(CHAINS * T, NB * W).astype(bf16)
        # inj tiles: round j uses inj_{S-j}; tile NB+1 = inj_{MEET}
        injv = xc * wvec[None, None, :]    # [128, S, T]
        mask_t = np.zeros((SEQ_PER_CORE, S), np.float32)
        mask_t[np.arange(SEQ_PER_CORE), tl] = 1.0
        injv = injv * mask_t[:, :, None]
        pinj = _pack_core(injv).reshape(CHAINS * T, S, W)
        steps_i = np.concatenate([np.arange(S - 1, MEET, -1), [MEET]])
        inj = np.ascontiguousarray(
            pinj[:, steps_i]).reshape(CHAINS * T, (NB + 1) * W).astype(bf16)
        in_maps.append({
            "xf": xf, "xb": xb, "inj": inj, "wmain": wmain, "wbc": wbc,
        })
    return in_maps


def _exp_factor(src):
    """Replay the device's clamped power-of-two renorm factor (f64)."""
    bits = np.ascontiguousarray(src.astype(np.float32)).view(np.uint32)
    bits = np.minimum(np.maximum(bits, CLAMP_LO), CLAMP_HI)
    gbits = (bits & np.uint32(0x7F800000)) ^ np.uint32(0x7F800000)
    return gbits.view(np.float32).astype(np.float64) * 0.5


def _denominators(res, a, tails):
    """Per-seq log partition from device outputs (f64 host replay)."""
    big_a = np.cumsum(a, axis=1)          # [B, S]
    nev = len(REN_EVENTS)
    ncap = len(CAP_ROUNDS)
    mvec = np.array(REN_EVENTS)           # event rounds [nev]
    den = np.zeros(B, np.float64)
    for core in range(NCORES):
        r = res.results[core]
        sl = slice(core * SEQ_PER_CORE, (core + 1) * SEQ_PER_CORE)
        t_b = tails[sl]                                    # [128]
        # [CHAINS, nev, W] -> [nev, 128]
        rsf = r["rsf"].astype(np.float64).reshape(CHAINS, nev, W)
        rsb = r["rsb"].astype(np.float64).reshape(CHAINS, nev, W)
        rsf = np.moveaxis(rsf, 1, 0).reshape(nev, SEQ_PER_CORE)
        rsb = np.moveaxis(rsb, 1, 0).reshape(nev, SEQ_PER_CORE)
        caps = r["caps"].astype(np.float64).reshape(CHAINS, ncap * 8, W)
        caps = caps.transpose(1, 0, 2).reshape(ncap * 8, SEQ_PER_CORE)
        qd = r["qdot"].astype(np.float64).reshape(SEQ_PER_CORE)

        lf = -np.log(_exp_factor(rsf))                     # [nev, 128]
        lb = -np.log(_exp_factor(rsb))
        long = t_b >= MEET
        # fwd offsets: all events for long; m <= tail for short
        use_f = long[None, :] | (mvec[:, None] <= t_b[None, :])
        off = np.sum(np.where(use_f, lf, 0.0), axis=0)
        # bwd offsets (long only): event processes step S-1-m
        use_b = long[None, :] & ((S - 1 - mvec)[:, None] < t_b[None, :])
        off += np.sum(np.where(use_b, lb, 0.0), axis=0)

        z_long = np.log(np.maximum(qd, 1e-300))
        idx = np.clip(t_b - CAP_BASE, 0, ncap * 8 - 1)
        z_short = np.log(np.maximum(caps[idx, np.arange(SEQ_PER_CORE)],
                                    1e-300))
        bidx = np.arange(SEQ_PER_CORE)
        den[sl] = (np.where(long, z_long, z_short)
                   + big_a[sl][bidx, t_b] + off)
    return den


def _numerator(em, tags, mask, startt, trans, endt):
    bsz, s_len, _ = em.shape
    tags = tags.astype(np.int64)
    ar = np.arange(s_len)
    bidx = np.arange(bsz)
    head = np.min(np.where(mask, ar[None, :], s_len - 1), axis=1)
    tail = np.max(ar[None, :] * mask, axis=1)
    nonempty = mask.sum(axis=1) != 0
    cond = mask[:, 1:] & (head[:, None] != ar[None, 1:])
    head_tags = tags[bidx, head]
    tail_tags = tags[bidx, tail]
    em64 = em.astype(np.float64)
    em_tag = np.take_along_axis(em64, tags[:, :, None], axis=2)[:, :, 0]
    trans_step = trans.astype(np.float64)[tags[:, :-1], tags[:, 1:]]
    num = (startt.astype(np.float64)[head_tags]
           + em_tag[bidx, head]
           + np.sum(np.where(cond, trans_step + em_tag[:, 1:], 0.0), axis=1)
           + endt.astype(np.float64)[tail_tags])
    return np.where(nonempty, num, 0.0)


def _finalize(den, num, mask):
    llh = den - num
    labels = mask.sum(axis=1).astype(np.float64)
    eps = 1e-6
    out = np.sum(llh / (labels + eps)) / (np.sum(labels != 0) + eps)
    return np.asarray(out, dtype=np.float32)


def kernel(**inputs):
    from concourse.bass_utils import run_bass_kernel_spmd

    em = np.asarray(inputs["emissions"], dtype=np.float32)
    tags = np.asarray(inputs["tags"])
    mask = np.asarray(inputs["mask"]).astype(bool)
    startt = np.asarray(inputs["start_transitions"], dtype=np.float32)
    trans = np.asarray(inputs["transitions"], dtype=np.float32)
    endt = np.asarray(inputs["end_transitions"], dtype=np.float32)
    bsz, s_len, t = em.shape
    assert (bsz, s_len, t) == (B, S, T), (bsz, s_len, t)

    ar = np.arange(s_len)
    tails = np.max(ar[None, :] * mask, axis=1)  # [B]

    x, a = _host_prep(em, startt)
    nc = _get_program()
    in_maps = _device_inputs(x, trans, endt, tails)
    res = run_bass_kernel_spmd(nc, in_maps, core_ids=list(range(NCORES)),
                               trace=TRACE)
    global LAST_RESULTS
    LAST_RESULTS = res

    den = _denominators(res, a, tails)
    num = _numerator(em, tags, mask, startt, trans, endt)
    return _finalize(den, num, mask)
